# revision 9
# baseline (speedup 1.0000x reference)
"""MeshConv (gnn_message_passing) Trainium2 Bass kernel, SPMD over 8 NeuronCores.

Per edge e with neighbor rows a0,a1,b0,b1 = x[neighbors[e, 0..3]] (zero row for
negative indices) and self row x[e]:
    desc_a = [a0+a1, |a0-a1|], desc_b = [b0+b1, |b0-b1|]
    out[e] = [x[e], desc_a+desc_b, |desc_a-desc_b|] @ W.T + b

Device formulation (W-folded to 192 features so the contraction splits into
K=128 + K=64 chunks):
    P=a0+a1, Q=b0+b1, R=a0-a1, S=b0-b1, Ra=|R|, Sa=|S|
    chunkA = [U1=P+Q, Ra, Sa, V1=|P-Q|] @ [W2;W3;W3;W4]^T   (K=128)
    chunkB = [V2=|Ra-Sa|, x]            @ [W5;W1]^T          (K=64)
    bias is added on the PSUM->SBUF output copy (DVE tensor add).

Edges are padded to 8*31*4096 and sharded contiguously across cores; within a
4096-edge block, edge (p,g) = base + 32*p + g so every DMA is >=2KB-contiguous
per partition. Neighbor rows and self rows are staged host-side in edge order
(the on-device indirect-DMA path on this stack only sustains 128 indices per
~1us instruction, an order of magnitude off the memory roofline), so the device
streams one [128, 20KB] tile per block, runs the combine on DVE/GPSIMD,
transposes feature chunks on the PE via identity matmuls, accumulates the two
K-chunks into PSUM, adds bias on the DVE output copy, and stores contiguously.
"""

import numpy as np

import concourse.bass as bass
import concourse.tile as tile
from concourse import bacc, mybir
from concourse.bass_utils import run_bass_kernel_spmd
from concourse.masks import make_identity

F32 = mybir.dt.float32

E = 1_000_000
C = 32
OUT = 64
NCORES = 8
G = 32                  # 128-edge groups per block
EPB = 128 * G           # edges per block = 4096
NBLK = 31               # blocks per core
SHARD = NBLK * EPB      # 126976 edges per core
E_PAD = NCORES * SHARD  # 1015808


def _build():
    nc = bacc.Bacc(
        "TRN2", target_bir_lowering=False, debug=False, num_devices=NCORES
    )
    # neighbor rows and self rows staged together: one DMA per block
    nbd = nc.dram_tensor("nbd", [NBLK, 128, G * 5 * C], F32, kind="ExternalInput").ap()
    wa = nc.dram_tensor("wa", [128, OUT], F32, kind="ExternalInput").ap()
    wb = nc.dram_tensor("wb", [128, OUT], F32, kind="ExternalInput").ap()
    bias8 = nc.dram_tensor("bias8", [128, 8 * OUT], F32, kind="ExternalInput").ap()
    out = nc.dram_tensor("out", [SHARD, OUT], F32, kind="ExternalOutput").ap()

    with tile.TileContext(nc) as tc:
        with (
            tc.tile_pool(name="consts", bufs=1) as consts,
            tc.tile_pool(name="nbp", bufs=2) as nbp,
            tc.tile_pool(name="tmp", bufs=2) as tmpp,
            tc.tile_pool(name="comba", bufs=2) as cap,
            tc.tile_pool(name="combb", bufs=2) as cbp,
            tc.tile_pool(name="cta", bufs=3) as ctap,
            tc.tile_pool(name="ctb", bufs=3) as ctbp,
            tc.tile_pool(name="outsb", bufs=2) as osp,
            tc.tile_pool(name="pa", bufs=2, space="PSUM") as pap,
            tc.tile_pool(name="pb", bufs=2, space="PSUM") as pbp,
            tc.tile_pool(name="po", bufs=2, space="PSUM") as pop,
        ):
            ident = consts.tile([128, 128], F32)
            make_identity(nc, ident[:])
            wa_sb = consts.tile([128, OUT], F32)
            nc.sync.dma_start(wa_sb[:], wa[:])
            wb_sb = consts.tile([128, OUT], F32)
            nc.sync.dma_start(wb_sb[:], wb[:])
            bias_sb = consts.tile([128, 8 * OUT], F32)
            nc.sync.dma_start(bias_sb[:], bias8[:])

            add = mybir.AluOpType.add
            sub = mybir.AluOpType.subtract
            band = mybir.AluOpType.bitwise_and
            I32 = mybir.dt.int32
            Abs = mybir.ActivationFunctionType.Abs

            for b in range(NBLK):
                nb_t = nbp.tile([128, G * 5 * C], F32)
                nc.sync.dma_start(nb_t[:], nbd[b])
                nbv = nb_t[:, : G * 4 * C].rearrange("p (g j d) -> p g j d", g=G, j=4)
                xsv = nb_t[:, G * 4 * C :].rearrange("p (g d) -> p g d", g=G)

                P = tmpp.tile([128, G, C], F32, tag="P")
                Q = tmpp.tile([128, G, C], F32, tag="Q")
                R = tmpp.tile([128, G, C], F32, tag="R")
                S = tmpp.tile([128, G, C], F32, tag="S")
                D1 = tmpp.tile([128, G, C], F32, tag="D1")
                D2 = tmpp.tile([128, G, C], F32, tag="D2")
                comba = cap.tile([128, G, 128], F32)
                combb = cbp.tile([128, G, 2 * C], F32)

                a0 = nbv[:, :, 0, :]
                a1 = nbv[:, :, 1, :]
                b0 = nbv[:, :, 2, :]
                b1 = nbv[:, :, 3, :]
                nc.vector.tensor_tensor(P[:], a0, a1, op=add)
                nc.vector.tensor_tensor(Q[:], b0, b1, op=add)
                nc.vector.tensor_tensor(R[:], a0, a1, op=sub)
                nc.vector.tensor_tensor(S[:], b0, b1, op=sub)
                # chunk A features: [U1 | Ra | Sa | V1]
                nc.vector.tensor_tensor(comba[:, :, 0:C], P[:], Q[:], op=add)
                nc.scalar.activation(comba[:, :, C : 2 * C], R[:], Abs)
                nc.scalar.activation(comba[:, :, 2 * C : 3 * C], S[:], Abs)
                nc.vector.tensor_tensor(D1[:], P[:], Q[:], op=sub)
                nc.scalar.activation(comba[:, :, 3 * C :], D1[:], Abs)
                # chunk B features: [V2 | x]
                nc.vector.tensor_tensor(
                    D2[:], comba[:, :, C : 2 * C], comba[:, :, 2 * C : 3 * C], op=sub
                )
                nc.scalar.activation(combb[:, :, 0:C], D2[:], Abs)
                nc.gpsimd.tensor_copy(combb[:, :, C:], xsv)

                # transpose chunk A: per group [128e,128f] -> [128f,128e]; 4/bank
                cta_tiles = []
                for q in range(G // 4):
                    pa_t = pap.tile([128, 512], F32)
                    for j in range(4):
                        g = 4 * q + j
                        nc.tensor.transpose(
                            pa_t[:, 128 * j : 128 * (j + 1)], comba[:, g, :], ident[:]
                        )
                    cta = ctap.tile([128, 512], F32)
                    nc.scalar.copy(cta[:], pa_t[:])
                    cta_tiles.append(cta)

                # transpose chunk B: group pairs [128e,128f] -> [128f,128e]; 4/bank
                ctb_tiles = []
                for h in range(G // 8):
                    pb_t = pbp.tile([128, 512], F32)
                    for u in range(4):
                        gp = 4 * h + u
                        nc.tensor.transpose(
                            pb_t[:, 128 * u : 128 * (u + 1)],
                            combb[:, 2 * gp : 2 * gp + 2, :],
                            ident[:],
                        )
                    ctb = ctbp.tile([128, 512], F32)
                    nc.scalar.copy(ctb[:], pb_t[:])
                    ctb_tiles.append(ctb)

                out_sb = osp.tile([128, G, OUT], F32)
                for ob in range(G // 8):
                    po_t = pop.tile([128, 512], F32)
                    for k in range(8):
                        g = 8 * ob + k
                        q, j = g // 4, g % 4
                        h, u, r = g // 8, (g % 8) // 2, g % 2
                        og = po_t[:, OUT * k : OUT * (k + 1)]
                        nc.tensor.matmul(
                            og,
                            lhsT=cta_tiles[q][:, 128 * j : 128 * (j + 1)],
                            rhs=wa_sb[:],
                            start=True,
                            stop=False,
                            skip_group_check=True,
                        )
                        nc.tensor.matmul(
                            og,
                            lhsT=ctb_tiles[h][
                                64 * r : 64 * (r + 1), 128 * u : 128 * (u + 1)
                            ],
                            rhs=wb_sb[64 * r : 64 * (r + 1), :],
                            start=False,
                            stop=True,
                            skip_group_check=True,
                        )
                    # bias folded into the PSUM->SBUF copy (DVE add)
                    nc.vector.tensor_tensor(
                        out_sb[:, 8 * ob : 8 * (ob + 1), :].rearrange("p g d -> p (g d)"),
                        po_t[:],
                        bias_sb[:],
                        op=add,
                    )

                nc.sync.dma_start(
                    out[b * EPB : (b + 1) * EPB].rearrange("(p g) d -> p g d", p=128),
                    out_sb[:],
                )

    nc.compile()
    return nc


_NC = None


def _get_nc():
    global _NC
    if _NC is None:
        _NC = _build()
    return _NC


def _host_prep(x, neighbors, W, b):
    x = np.ascontiguousarray(np.asarray(x, dtype=np.float32))
    neighbors = np.asarray(neighbors)
    W = np.asarray(W, dtype=np.float32)
    b = np.asarray(b, dtype=np.float32)
    assert x.shape == (E, C) and neighbors.shape == (E, 4)

    xg = np.concatenate([x, np.zeros((1, C), np.float32)], axis=0)  # zero row at E

    nb_pad = np.full((E_PAD, 4), E, dtype=np.int64)
    nb_pad[: neighbors.shape[0]] = neighbors
    nb_pad = np.where(nb_pad < 0, E, nb_pad)
    xs_pad = np.zeros((E_PAD, C), np.float32)
    xs_pad[: x.shape[0]] = x

    # W = [W1|W2|W3|W4|W5] along the 5C input features
    W1, W2, W3, W4, W5 = (W[:, i * C : (i + 1) * C].T.copy() for i in range(5))
    wa = np.concatenate([W2, W3, W3, W4], axis=0).astype(np.float32)
    wb = np.concatenate([W5, W1, W5, W1], axis=0).astype(np.float32)
    bias8 = np.broadcast_to(np.tile(b, 8), (128, 8 * OUT)).copy().astype(np.float32)

    in_maps = []
    for c in range(NCORES):
        lo, hi = c * SHARD, (c + 1) * SHARD
        # edge (blk, p, g) = lo + blk*EPB + 32p + g
        nbr = xg[nb_pad[lo:hi].ravel()].reshape(NBLK, 128, G * 4 * C)
        xsr = xs_pad[lo:hi].reshape(NBLK, 128, G * C)
        nbd = np.concatenate([nbr, xsr], axis=2)
        in_maps.append(
            {
                "nbd": nbd,
                "wa": wa,
                "wb": wb,
                "bias8": bias8,
            }
        )

    return in_maps


def kernel(x, neighbors, W, b):
    n_edges = np.asarray(neighbors).shape[0]
    nc = _get_nc()
    in_maps = _host_prep(x, neighbors, W, b)
    res = run_bass_kernel_spmd(nc, in_maps, core_ids=list(range(NCORES)))
    outs = [r["out"] for r in res.results]
    return np.concatenate(outs, axis=0)[:n_edges]
